# revision 46
# baseline (speedup 1.0000x reference)
"""TRN2 Bass kernel for nn_Attention_Attention_85272280695427.

Dense transformer: mlp_in -> feature-attention(768) -> mlp -> feature
-attention(384) -> mean-pool -> mlp_out.  Pure data parallel over the
batch dim B=256 across 8 NeuronCores (32 batches/core); weights
replicated.

Design notes (all matmuls float32r = full PE rate, ~1e-4 precision):
  * Activations kept feature-major [D, n] or agent-major [n, D] so every
    matmul contracts over the partition dim; weights pre-transposed on
    the host.
  * silu(x) computed as (1 + tanh(x/2)) * x = 2*silu(x) -- tanh lives in
    the same ScalarE table set as exp, so the kernel never switches
    activation table sets.  The factor 2 is absorbed into the next
    layer's (host-prescaled) weights; the resulting 4x on S = Q@K.T is
    undone with the exp activation's free scale=0.25.
  * Softmax (over e): compute ST[e,d] = S.T directly; ET = exp(S-C)
    with fixed shift constants C4/C7 (safe for this model's S ranges);
    denominator Z arrives for free as two extra "4.0"-columns appended
    to V; 1/(4Z) feeds the epilogue tanh's per-partition scale.
  * Biases are injected as K=1 matmul steps (ones x bias-row).
  * Mean-pool via the epilogue's accum_out; final [32,128] output
    produced by one fp32 matmul over all batches.
"""

import sys

if "/opt/trn_rl_repo" not in sys.path:
    sys.path.insert(0, "/opt/trn_rl_repo")

import numpy as np
from contextlib import ExitStack

import concourse.bass as bass
import concourse.tile as tile
from concourse import bacc, mybir
from concourse.bass_utils import run_bass_kernel_spmd

F32 = mybir.dt.float32
F32R = mybir.dt.float32r
AF = mybir.ActivationFunctionType
OP = mybir.AluOpType

B, NA, IN_DIM, H, OUT_DIM = 256, 256, 256, 384, 128
D2 = 2 * H  # 768
NCORES = 8
B_LOC = B // NCORES  # 32
C4 = 60.0  # softmax shift, block 4 (S4 in [-33, 118] across input variants)
C7 = 12.0  # softmax shift, block 7 (S7 in [-9, 25])

_NC_CACHE = {}


def sl(t, w=128):
    return slice(t * w, (t + 1) * w)


def build(nc, b_loc=B_LOC):
    din = lambda n, s, dt=F32R: nc.dram_tensor(n, s, dt, kind="ExternalInput").ap()

    xt = din("xt", [b_loc, IN_DIM, NA])
    wit = din("wit", [IN_DIM, D2])
    aqkt = din("aqkt", [D2, 2 * D2])
    avt = din("avt", [D2, D2])
    w4t = din("w4t", [D2, H])
    aq7t = din("aq7t", [H, H])
    ak7t = din("ak7t", [H, H])
    av7t = din("av7t", [H, H])
    wot32 = din("wot32", [H, OUT_DIM], F32)
    # bias rows packed at base partitions 0/32 (matmul base-partition rule)
    brda = din("brda", [33, 1024])  # r0: bqk[0:1024]; r32: bqk[1024:1536]
    brh = din("brh", [33, H])       # r0: bq7; r32: bk7
    ones2 = din("ones2", [33, 128])  # ones rows at partitions 0 and 32
    borow32 = din("borow32", [1, OUT_DIM], F32)
    ones32 = din("ones32", [1, b_loc], F32)
    vcol = din("vcol", [128, 12])  # 4.0-valued Z columns for V tiles
    bcols = din("bcols", [128, 18], F32)  # bias columns: bi|bv4|b4|bv7

    out = nc.dram_tensor("out", [b_loc, OUT_DIM], F32, kind="ExternalOutput").ap()

    with tile.TileContext(nc) as tc, ExitStack() as ctx:
        cp = ctx.enter_context(tc.tile_pool(name="const", bufs=1))
        xp = ctx.enter_context(tc.tile_pool(name="xfm", bufs=1))
        hp = ctx.enter_context(tc.tile_pool(name="hfm", bufs=1))
        qkp = ctx.enter_context(tc.tile_pool(name="qkt4", bufs=2))
        ep = ctx.enter_context(tc.tile_pool(name="et4", bufs=2))
        o4p = ctx.enter_context(tc.tile_pool(name="o4", bufs=6))
        h5p = ctx.enter_context(tc.tile_pool(name="h5", bufs=1))
        q7p = ctx.enter_context(tc.tile_pool(name="qt7", bufs=2))
        k7p = ctx.enter_context(tc.tile_pool(name="kt7", bufs=2))
        tp = ctx.enter_context(tc.tile_pool(name="tpool", bufs=4))
        x2p = ctx.enter_context(tc.tile_pool(name="x2", bufs=2))
        rp = ctx.enter_context(tc.tile_pool(name="recip", bufs=6))
        ps_sm = ctx.enter_context(tc.tile_pool(name="ps_sm", bufs=8, space="PSUM"))

        def silu2(out_ap, psum_ap, w):
            """out = (1 + tanh(psum/2)) * psum = 2*silu(psum)."""
            t = tp.tile([128, 512], F32, tag="t")
            nc.scalar.activation(t[:, 0:w], psum_ap, AF.Tanh, scale=0.5)
            nc.vector.scalar_tensor_tensor(out_ap, t[:, 0:w], 1.0, psum_ap,
                                           OP.add, OP.mult)

        # ---- constants / weights (resident) ----
        def ctile(shape, src, dt=F32R, tag=None):
            t = cp.tile(shape, dt, tag=tag)
            nc.sync.dma_start(t[:], src)
            return t

        wit_t = ctile([128, 2, D2], wit.rearrange("(t p) d -> p t d", p=128), tag="wit")
        aqkt_t = cp.tile([128, 6, 2 * D2], F32R, tag="aqkt")
        for c in range(3):
            nc.sync.dma_start(
                aqkt_t[:, :, 512 * c:512 * (c + 1)],
                aqkt[:, 512 * c:512 * (c + 1)].rearrange(
                    "(t p) d -> p t d", p=128))
        avt_t = cp.tile([128, 6, D2], F32R, tag="avt")
        for c in range(2):
            nc.sync.dma_start(
                avt_t[:, :, 384 * c:384 * (c + 1)],
                avt[:, 384 * c:384 * (c + 1)].rearrange("(t p) d -> p t d", p=128))
        w4t_t = ctile([128, 6, H], w4t.rearrange("(t p) d -> p t d", p=128), tag="w4t")
        aq7t_t = ctile([128, 3, H], aq7t.rearrange("(t p) d -> p t d", p=128), tag="aq7t")
        ak7t_t = ctile([128, 3, H], ak7t.rearrange("(t p) d -> p t d", p=128), tag="ak7t")
        av7t_t = ctile([128, 3, H], av7t.rearrange("(t p) d -> p t d", p=128), tag="av7t")
        brda_t = ctile([33, 1024], brda[:], tag="brda")
        brh_t = ctile([33, H], brh[:], tag="brh")
        bcols_t = ctile([128, 18], bcols[:], F32, tag="bcols")
        ones2_t = ctile([33, 128], ones2[:], tag="ones2")
        borow_t = ctile([1, OUT_DIM], borow32[:], F32, tag="borow")
        ones32_t = ctile([1, b_loc], ones32[:], F32, tag="ones32")
        bq7row, bk7row = brh_t[0:1, 0:H], brh_t[32:33, 0:H]
        ones_a, ones_b = ones2_t[0:1, :], ones2_t[32:33, :]
        # merged-QK chunk biases: (row-slice, ones-row) per 512-chunk
        qkbias = ((brda_t[0:1, 0:512], ones_a),
                  (brda_t[0:1, 512:1024], ones_a),
                  (brda_t[32:33, 0:512], ones_b))

        # persistent paired V tiles; per e_t: [A(256) Z(2) pad | B(256) Z(2) pad]
        v4_t = cp.tile([128, 6, 520], F32R, tag="v4p")
        v7_t = cp.tile([128, 3, 520], F32R, tag="v7p")
        for off in (256, 516):
            nc.sync.dma_start(v4_t[:, :, off:off + 2],
                              vcol[:, 0:12].rearrange("p (t d) -> p t d", t=6))
            nc.sync.dma_start(v7_t[:, :, off:off + 2],
                              vcol[:, 0:6].rearrange("p (t d) -> p t d", t=3))

        pooled = cp.tile([128, 3, b_loc], F32, tag="pooled")
        nc4 = cp.tile([128, 1], F32, tag="nc4")
        nc.gpsimd.memset(nc4[:], -C4)
        nc7 = cp.tile([128, 1], F32, tag="nc7")
        nc.gpsimd.memset(nc7[:], -C7)

        MM = nc.tensor.matmul

        state = {}

        def h1_groups(pr):
            """Dense front half for pair (2pr, 2pr+1): mlp_in/V4 run both
            batches at N=512; Q/K per batch-half.  Returns (stage_a, stage_b)
            emit-thunk lists; stage_b must be emitted after the previous
            pair's B-half ST4 reads (slot-reuse order)."""
            bA = 2 * pr
            xfm = xp.tile([128, 2, 2, NA], F32R, tag="xfm")
            hfm = hp.tile([128, 6, 512], F32R, tag="hfm")
            qkt = [qkp.tile([128, 2, 2 * D2], F32R, tag="qkt4", name="qkt4")
                   for _ in range(2)]
            for h in range(2):
                state[bA + h] = dict(qkt=qkt[h], voff=260 * h)
            ga, gb = [], []

            def g_x():
                for h in range(2):
                    nc.sync.dma_start(
                        xfm[:, :, h, :],
                        xt[bA + h].rearrange("(t p) n -> p t n", p=128))
            ga.append(g_x)

            def g_mlp(d_t):
                p = ps_sm.tile([128, 512], F32, tag="psm")
                MM(p[:, 0:512], wit_t[:, 0, sl(d_t)], xfm[:, 0, :, :],
                   start=True, stop=False)
                MM(p[:, 0:512], wit_t[:, 1, sl(d_t)], xfm[:, 1, :, :],
                   start=False, stop=True)
                nc.vector.tensor_scalar_add(p[:, 0:512], p[:, 0:512],
                                            bcols_t[:, d_t:d_t + 1])
                silu2(hfm[:, d_t, :], p[:, 0:512], 512)
            for d_t in range(6):
                ga.append(lambda d_t=d_t: g_mlp(d_t))

            def g_qk(dst, h, n_t):
                ps = [ps_sm.tile([128, 512], F32, tag="psm", name="pqk")
                      for _ in range(3)]
                for c in range(3):
                    brow, o1 = qkbias[c]
                    MM(ps[c][:], o1[:, 0:128], brow[:],
                       start=True, stop=False)
                for e_t in range(6):
                    for c in range(3):  # same stationary 3x (ldw-opt)
                        MM(ps[c][:], hfm[:, e_t, sl(2 * h + n_t)],
                           aqkt_t[:, e_t, 512 * c:512 * (c + 1)],
                           start=False, stop=(e_t == 5))
                for c in range(3):
                    silu2(dst[:, n_t, 512 * c:512 * (c + 1)], ps[c][:], 512)
            for h, lst in ((0, ga), (1, gb)):
                for n_t in range(2):
                    lst.append(lambda a=qkt[h], hh=h, n=n_t: g_qk(a, hh, n))

            def g_v4(d_t):
                p = ps_sm.tile([128, 512], F32, tag="psm")
                for e_t in range(6):
                    MM(p[:, 0:512], avt_t[:, e_t, sl(d_t)], hfm[:, e_t, :],
                       start=(e_t == 0), stop=(e_t == 5))
                nc.vector.tensor_scalar_add(p[:, 0:512], p[:, 0:512],
                                            bcols_t[:, 6 + d_t:7 + d_t])
                t = tp.tile([128, 512], F32, tag="t")
                nc.scalar.activation(t[:], p[:, 0:512], AF.Tanh, scale=0.5)
                vo = v4_t[:, d_t, 0:520].rearrange(
                    "p (b x) -> p b x", x=260)[:, :, 0:256]
                nc.vector.scalar_tensor_tensor(
                    vo, t[:].rearrange("p (b x) -> p b x", x=256), 1.0,
                    p[:, 0:512].rearrange("p (b x) -> p b x", x=256),
                    OP.add, OP.mult)
            for d_t in range(6):
                gb.append(lambda d_t=d_t: g_v4(d_t))
            return ga, gb

        def h2_groups(pr):
            """Sparse back half for pair pr: per-batch attention4, then paired
            mlp4/h5/V7, per-batch block7 attention + pooling."""
            o4s = [o4p.tile([128, 512], F32R, tag="o4", name="o4")
                   for _ in range(6)]
            h5 = h5p.tile([128, 3, 512], F32R, tag="h5")
            qk7 = [[q7p.tile([128, 2, H], F32R, tag="qt7", name="qt7"),
                    k7p.tile([128, 2, H], F32R, tag="kt7", name="kt7")]
                   for _ in range(2)]
            sts = [state.pop(2 * pr), state.pop(2 * pr + 1)]

            # --- per-batch block4 attention ---
            def g_st(hb, e_t, et4, hc):
                qkt4 = sts[hb]["qkt"]
                p = ps_sm.tile([128, 384], F32, tag="psm")
                for n_t in range(2):
                    MM(p[:, 0:384],
                       qkt4[:, n_t, D2 + e_t * 128:D2 + (e_t + 1) * 128],
                       qkt4[:, n_t, hc:hc + 384],
                       start=(n_t == 0), stop=(n_t == 1))
                nc.scalar.activation(et4[:, e_t, :], p[:, 0:384], AF.Exp,
                                     bias=nc4[:, 0:1], scale=0.25)

            def g_av(hb, d_t, et4, half):
                voff = sts[hb]["voff"]
                d_g = half * 3 + d_t
                p = ps_sm.tile([128, 384], F32, tag="psm")
                for e_t in range(6):
                    MM(p[:, 0:258], et4[:, e_t, sl(d_t)],
                       v4_t[:, e_t, voff:voff + 258],
                       start=(e_t == 0), stop=(e_t == 5))
                rho = rp.tile([128, 1], F32, tag="rho")
                nc.vector.reciprocal(rho[:], p[:, 256:257])
                t = tp.tile([128, 512], F32, tag="t")
                nc.scalar.activation(t[:, 0:NA], p[:, 0:NA], AF.Tanh,
                                     scale=rho[:, 0:1])
                x2 = x2p.tile([128, NA], F32, tag="x2")
                nc.vector.tensor_scalar(x2[:], p[:, 0:NA], rho[:, 0:1],
                                        2.0, OP.mult, OP.mult)
                nc.vector.scalar_tensor_tensor(
                    o4s[d_g][:, 256 * hb:256 * hb + 256], t[:, 0:NA], 1.0,
                    x2[:], OP.add, OP.mult)

            for hb in range(2):
                et4s = [ep.tile([128, 6, 384], F32R, tag="et4", name="et4")
                        for _ in range(2)]
                for half in range(2):
                    for e_t in range(6):
                        yield (lambda hb=hb, e_t=e_t, et4=et4s[half],
                               hc=half * 384: g_st(hb, e_t, et4, hc))
                    for d_t in range(3):
                        yield (lambda hb=hb, d_t=d_t, et4=et4s[half],
                               half=half: g_av(hb, d_t, et4, half))

            # --- paired mlp4 -> h5 ---
            def g_mlp4(h_t):
                p = ps_sm.tile([128, 512], F32, tag="psm")
                for d_g in range(6):
                    MM(p[:], w4t_t[:, d_g, sl(h_t)], o4s[d_g][:],
                       start=(d_g == 0), stop=(d_g == 5))
                nc.vector.tensor_scalar_add(p[:], p[:],
                                            bcols_t[:, 12 + h_t:13 + h_t])
                silu2(h5[:, h_t, :], p[:], 512)
            for h_t in range(3):
                yield (lambda h_t=h_t: g_mlp4(h_t))

            # --- per-batch Q7/K7 (stationary = h5 pair slices) ---
            def g_qk7(hb, n_t):
                pq = ps_sm.tile([128, 384], F32, tag="psm", name="pq7")
                pk = ps_sm.tile([128, 384], F32, tag="psm", name="pk7")
                MM(pq[:, 0:H], ones_a[:, 0:128], bq7row[:, 0:H],
                   start=True, stop=False)
                MM(pk[:, 0:H], ones_b[:, 0:128], bk7row[:, 0:H],
                   start=True, stop=False)
                for h_t in range(3):  # same stationary 2x (ldw-opt)
                    MM(pq[:, 0:H], h5[:, h_t, sl(2 * hb + n_t)],
                       aq7t_t[:, h_t, 0:H], start=False, stop=(h_t == 2))
                    MM(pk[:, 0:H], h5[:, h_t, sl(2 * hb + n_t)],
                       ak7t_t[:, h_t, 0:H], start=False, stop=(h_t == 2))
                silu2(qk7[hb][0][:, n_t, :], pq[:, 0:H], H)
                silu2(qk7[hb][1][:, n_t, :], pk[:, 0:H], H)
            for hb in range(2):
                for n_t in range(2):
                    yield (lambda hh=hb, n=n_t: g_qk7(hh, n))

            # --- paired V7 ---
            def g_v7(d_t):
                p = ps_sm.tile([128, 512], F32, tag="psm")
                for h_t in range(3):
                    MM(p[:], av7t_t[:, h_t, sl(d_t)], h5[:, h_t, :],
                       start=(h_t == 0), stop=(h_t == 2))
                nc.vector.tensor_scalar_add(p[:], p[:],
                                            bcols_t[:, 15 + d_t:16 + d_t])
                t = tp.tile([128, 512], F32, tag="t")
                nc.scalar.activation(t[:], p[:], AF.Tanh, scale=0.5)
                vo = v7_t[:, d_t, 0:520].rearrange(
                    "p (b x) -> p b x", x=260)[:, :, 0:256]
                nc.vector.scalar_tensor_tensor(
                    vo, t[:].rearrange("p (b x) -> p b x", x=256), 1.0,
                    p[:].rearrange("p (b x) -> p b x", x=256),
                    OP.add, OP.mult)
            for d_t in range(3):
                yield (lambda d_t=d_t: g_v7(d_t))

            # --- per-batch block7 attention + pooling ---
            def g_st7(hb, e_t, et7):
                p = ps_sm.tile([128, 384], F32, tag="psm")
                for n_t in range(2):
                    MM(p[:, 0:H], qk7[hb][1][:, n_t, sl(e_t)],
                       qk7[hb][0][:, n_t, 0:H],
                       start=(n_t == 0), stop=(n_t == 1))
                nc.scalar.activation(et7[:, e_t, :], p[:, 0:H], AF.Exp,
                                     bias=nc7[:, 0:1], scale=0.25)

            def g_av7(hb, d_t, et7):
                voff = sts[hb]["voff"]
                b = 2 * pr + hb
                p = ps_sm.tile([128, 384], F32, tag="psm")
                for e_t in range(3):
                    MM(p[:, 0:258], et7[:, e_t, sl(d_t)],
                       v7_t[:, e_t, voff:voff + 258],
                       start=(e_t == 0), stop=(e_t == 2))
                rho = rp.tile([128, 1], F32, tag="rho")
                nc.vector.reciprocal(rho[:], p[:, 256:257])
                t = tp.tile([128, 512], F32, tag="t")
                nc.scalar.activation(t[:, 0:NA], p[:, 0:NA], AF.Tanh,
                                     scale=rho[:, 0:1])
                x2 = x2p.tile([128, NA], F32, tag="x2")
                nc.vector.tensor_scalar(x2[:], p[:, 0:NA], rho[:, 0:1], 2.0,
                                        OP.mult, OP.mult)
                nc.vector.scalar_tensor_tensor(
                    x2[:], t[:, 0:NA], 1.0, x2[:], OP.add, OP.mult,
                    accum_out=pooled[:, d_t, b:b + 1])

            for hb in range(2):
                et7 = ep.tile([128, 6, 384], F32R, tag="et4", name="et7")[:, 0:3, :]
                for e_t in range(3):
                    yield (lambda hb=hb, e_t=e_t, et7=et7: g_st7(hb, e_t, et7))
                for d_t in range(3):
                    yield (lambda hb=hb, d_t=d_t, et7=et7: g_av7(hb, d_t, et7))

        # Software-pipelined emission: pair pr's dense front half is
        # interleaved with pair pr-1's sparse back half (HAM duty), with
        # staging so every tile slot's readers precede its next writer in
        # per-engine emission order (else the scheduler deadlocks):
        #   stage A groups ride along g2[0:40]   (covers prev A's ST4)
        #   g2[40:49] emitted solo               (prev B's ST4)
        #   stage B groups ride along g2[49:]    (prev B's AV4 done early)
        def merge(g2part, g1part):
            i1 = i2 = 0
            n1, n2 = len(g1part), len(g2part)
            while i1 < n1 or i2 < n2:
                if i2 < n2 and (i1 >= n1 or i2 * n1 <= i1 * n2):
                    g2part[i2](); i2 += 1
                else:
                    g1part[i1](); i1 += 1

        n_pairs = b_loc // 2
        for pr in range(n_pairs + 1):
            ga, gb = h1_groups(pr) if pr < n_pairs else ([], [])
            g2 = list(h2_groups(pr - 1)) if pr >= 1 else []
            # stage so slot readers precede next writers in emission order:
            #  ga head (x, mlp_in) rides A-attn; qkA rides B-attn-head (needs
            #  prev A-ST4 done at g2[14]); qkB+V4 ride the tail (need prev
            #  B-ST4 at g2[32] / AV4 done at g2[35]).
            merge(g2[0:18], ga[0:7])
            merge(g2[18:33], ga[7:])
            for g in g2[33:36]:
                g()
            merge(g2[36:], gb)

        # ---------- final: out = silu(pooled^T Wo^T/(2*256) + bo) ----------
        wot_t = xp.tile([128, 3, OUT_DIM], F32, tag="xfm", name="wot")
        nc.sync.dma_start(wot_t[:], wot32.rearrange("(t p) d -> p t d", p=128))
        po = ps_sm.tile([b_loc, OUT_DIM], F32, tag="psm")
        for h_t in range(3):
            MM(po[:, :], pooled[:, h_t, :], wot_t[:, h_t, :],
               start=(h_t == 0), stop=False)
        MM(po[:, :], ones32_t[0:1, 0:b_loc], borow_t[0:1, 0:OUT_DIM],
           start=False, stop=True)
        out_sb = xp.tile([b_loc, OUT_DIM], F32, tag="xfm", name="out_sb")
        nc.scalar.activation(out_sb[:], po[:, :], AF.Silu)
        nc.sync.dma_start(out[:], out_sb[:])

    nc.compile()
    return nc


def _get_nc(b_loc=B_LOC):
    if b_loc not in _NC_CACHE:
        nc = bacc.Bacc("TRN2", target_bir_lowering=False, debug=False,
                       num_devices=NCORES)
        _NC_CACHE[b_loc] = build(nc, b_loc)
    return _NC_CACHE[b_loc]


def make_in_maps(inputs, b_loc=B_LOC):
    f = np.float32
    asT = lambda k: np.ascontiguousarray(np.asarray(inputs[k], f).T)
    row = lambda k, d: np.asarray(inputs[k], f).reshape(1, d)
    x = np.asarray(inputs["x"], f)
    def pack2(r0, r32, d):
        a = np.zeros((33, d), f)
        a[0] = r0.ravel(); a[32] = r32.ravel()
        return a
    bqk = np.concatenate([np.asarray(inputs["Bq4"], f).ravel(),
                          np.asarray(inputs["Bk4"], f).ravel()])
    brda = np.zeros((33, 1024), f)
    brda[0] = bqk[0:1024]
    brda[32, 0:512] = bqk[1024:1536]
    brh = np.zeros((33, H), f)
    brh[0] = np.asarray(inputs["Bq7"], f).ravel()
    brh[32] = np.asarray(inputs["Bk7"], f).ravel()
    ones2 = np.zeros((33, 128), f); ones2[0] = 1.0; ones2[32] = 1.0
    consts = dict(
        wit=asT("Wi"),
        aqkt=np.concatenate([asT("Aq4"), asT("Ak4")], axis=1) * 0.5,
        avt=asT("Av4") * 0.5,
        w4t=asT("W4") * 0.5,
        aq7t=asT("Aq7") * 0.5, ak7t=asT("Ak7") * 0.5, av7t=asT("Av7") * 0.5,
        wot32=asT("Wo") / (2.0 * NA),
        brda=brda, brh=brh, ones2=ones2,
        borow32=row("bo", OUT_DIM),
        ones32=np.ones((1, b_loc), f),
        vcol=np.full((128, 12), 4.0, f),
        bcols=np.concatenate([
            np.asarray(inputs["bi"], f).reshape(6, 128).T,
            np.asarray(inputs["Bv4"], f).reshape(6, 128).T,
            np.asarray(inputs["b4"], f).reshape(3, 128).T,
            np.asarray(inputs["Bv7"], f).reshape(3, 128).T], axis=1),
    )
    in_maps = []
    for c in range(NCORES):
        xb = x[c * b_loc:(c + 1) * b_loc]
        m = dict(consts)
        m["xt"] = np.ascontiguousarray(xb.transpose(0, 2, 1))
        in_maps.append(m)
    return in_maps


def kernel(**inputs) -> np.ndarray:
    nc = _get_nc()
    in_maps = make_in_maps(inputs)
    res = run_bass_kernel_spmd(nc, in_maps, core_ids=list(range(NCORES)))
    out = np.concatenate([r["out"] for r in res.results], axis=0)
    return out.reshape(B // NA, NA, OUT_DIM).astype(np.float32)


# revision 47
# speedup vs baseline: 1.0130x; 1.0130x over previous
"""TRN2 Bass kernel for nn_Attention_Attention_85272280695427.

Dense transformer: mlp_in -> feature-attention(768) -> mlp -> feature
-attention(384) -> mean-pool -> mlp_out.  Pure data parallel over the
batch dim B=256 across 8 NeuronCores (32 batches/core); weights
replicated.

Design notes (all matmuls float32r = full PE rate, ~1e-4 precision):
  * Activations kept feature-major [D, n] or agent-major [n, D] so every
    matmul contracts over the partition dim; weights pre-transposed on
    the host.
  * silu(x) computed as (1 + tanh(x/2)) * x = 2*silu(x) -- tanh lives in
    the same ScalarE table set as exp, so the kernel never switches
    activation table sets.  The factor 2 is absorbed into the next
    layer's (host-prescaled) weights; the resulting 4x on S = Q@K.T is
    undone with the exp activation's free scale=0.25.
  * Softmax (over e): compute ST[e,d] = S.T directly; ET = exp(S-C)
    with fixed shift constants C4/C7 (safe for this model's S ranges);
    denominator Z arrives for free as two extra "4.0"-columns appended
    to V; 1/(4Z) feeds the epilogue tanh's per-partition scale.
  * Biases are injected as K=1 matmul steps (ones x bias-row).
  * Mean-pool via the epilogue's accum_out; final [32,128] output
    produced by one fp32 matmul over all batches.
"""

import sys

if "/opt/trn_rl_repo" not in sys.path:
    sys.path.insert(0, "/opt/trn_rl_repo")

import numpy as np
from contextlib import ExitStack

import concourse.bass as bass
import concourse.tile as tile
from concourse import bacc, mybir
from concourse.bass_utils import run_bass_kernel_spmd

F32 = mybir.dt.float32
F32R = mybir.dt.float32r
AF = mybir.ActivationFunctionType
OP = mybir.AluOpType

B, NA, IN_DIM, H, OUT_DIM = 256, 256, 256, 384, 128
D2 = 2 * H  # 768
NCORES = 8
B_LOC = B // NCORES  # 32
C4 = 60.0  # softmax shift, block 4 (S4 in [-33, 118] across input variants)
C7 = 12.0  # softmax shift, block 7 (S7 in [-9, 25])

_NC_CACHE = {}


def sl(t, w=128):
    return slice(t * w, (t + 1) * w)


def build(nc, b_loc=B_LOC):
    din = lambda n, s, dt=F32R: nc.dram_tensor(n, s, dt, kind="ExternalInput").ap()

    xt = din("xt", [b_loc, IN_DIM, NA])
    wit = din("wit", [IN_DIM, D2])
    aqkt = din("aqkt", [D2, 2 * D2])
    avt = din("avt", [D2, D2])
    w4t = din("w4t", [D2, H])
    aq7t = din("aq7t", [H, H])
    ak7t = din("ak7t", [H, H])
    av7t = din("av7t", [H, H])
    wot32 = din("wot32", [H, OUT_DIM], F32)
    # bias rows packed at base partitions 0/32 (matmul base-partition rule)
    brda = din("brda", [33, 1024])  # r0: bqk[0:1024]; r32: bqk[1024:1536]
    brh = din("brh", [33, H])       # r0: bq7; r32: bk7
    ones2 = din("ones2", [33, 128])  # ones rows at partitions 0 and 32
    borow32 = din("borow32", [1, OUT_DIM], F32)
    ones32 = din("ones32", [1, b_loc], F32)
    vcol = din("vcol", [128, 12])  # 4.0-valued Z columns for V tiles
    bcols = din("bcols", [128, 18], F32)  # bias columns: bi|bv4|b4|bv7

    out = nc.dram_tensor("out", [b_loc, OUT_DIM], F32, kind="ExternalOutput").ap()

    with tile.TileContext(nc) as tc, ExitStack() as ctx:
        cp = ctx.enter_context(tc.tile_pool(name="const", bufs=1))
        xp = ctx.enter_context(tc.tile_pool(name="xfm", bufs=1))
        hp = ctx.enter_context(tc.tile_pool(name="hfm", bufs=1))
        qkp = ctx.enter_context(tc.tile_pool(name="qkt4", bufs=2))
        ep = ctx.enter_context(tc.tile_pool(name="et4", bufs=2))
        o4p = ctx.enter_context(tc.tile_pool(name="o4", bufs=6))
        h5p = ctx.enter_context(tc.tile_pool(name="h5", bufs=1))
        q7p = ctx.enter_context(tc.tile_pool(name="qt7", bufs=2))
        k7p = ctx.enter_context(tc.tile_pool(name="kt7", bufs=2))
        tp = ctx.enter_context(tc.tile_pool(name="tpool", bufs=3))
        x2p = ctx.enter_context(tc.tile_pool(name="x2", bufs=2))
        rp = ctx.enter_context(tc.tile_pool(name="recip", bufs=6))
        ps_sm = ctx.enter_context(tc.tile_pool(name="ps_sm", bufs=8, space="PSUM"))

        def silu2(out_ap, psum_ap, w):
            """out = (1 + tanh(psum/2)) * psum = 2*silu(psum)."""
            t = tp.tile([128, 512], F32, tag="t")
            nc.scalar.activation(t[:, 0:w], psum_ap, AF.Tanh, scale=0.5)
            nc.vector.scalar_tensor_tensor(out_ap, t[:, 0:w], 1.0, psum_ap,
                                           OP.add, OP.mult)

        # ---- constants / weights (resident) ----
        def ctile(shape, src, dt=F32R, tag=None):
            t = cp.tile(shape, dt, tag=tag)
            nc.sync.dma_start(t[:], src)
            return t

        wit_t = ctile([128, 2, D2], wit.rearrange("(t p) d -> p t d", p=128), tag="wit")
        aqkt_t = cp.tile([128, 6, 2 * D2], F32R, tag="aqkt")
        for c in range(3):
            nc.sync.dma_start(
                aqkt_t[:, :, 512 * c:512 * (c + 1)],
                aqkt[:, 512 * c:512 * (c + 1)].rearrange(
                    "(t p) d -> p t d", p=128))
        avt_t = cp.tile([128, 6, D2], F32R, tag="avt")
        for c in range(2):
            nc.sync.dma_start(
                avt_t[:, :, 384 * c:384 * (c + 1)],
                avt[:, 384 * c:384 * (c + 1)].rearrange("(t p) d -> p t d", p=128))
        w4t_t = ctile([128, 6, H], w4t.rearrange("(t p) d -> p t d", p=128), tag="w4t")
        aq7t_t = ctile([128, 3, H], aq7t.rearrange("(t p) d -> p t d", p=128), tag="aq7t")
        ak7t_t = ctile([128, 3, H], ak7t.rearrange("(t p) d -> p t d", p=128), tag="ak7t")
        av7t_t = ctile([128, 3, H], av7t.rearrange("(t p) d -> p t d", p=128), tag="av7t")
        brda_t = ctile([33, 1024], brda[:], tag="brda")
        brh_t = ctile([33, H], brh[:], tag="brh")
        bcols_t = ctile([128, 18], bcols[:], F32, tag="bcols")
        ones2_t = ctile([33, 128], ones2[:], tag="ones2")
        borow_t = ctile([1, OUT_DIM], borow32[:], F32, tag="borow")
        ones32_t = ctile([1, b_loc], ones32[:], F32, tag="ones32")
        bq7row, bk7row = brh_t[0:1, 0:H], brh_t[32:33, 0:H]
        ones_a, ones_b = ones2_t[0:1, :], ones2_t[32:33, :]
        # merged-QK chunk biases: (row-slice, ones-row) per 512-chunk
        qkbias = ((brda_t[0:1, 0:512], ones_a),
                  (brda_t[0:1, 512:1024], ones_a),
                  (brda_t[32:33, 0:512], ones_b))

        # persistent paired V tiles; per e_t: [A(256) Z(2) pad | B(256) Z(2) pad]
        v4_t = cp.tile([128, 6, 520], F32R, tag="v4p")
        v7_t = cp.tile([128, 3, 520], F32R, tag="v7p")
        for off in (256, 516):
            nc.sync.dma_start(v4_t[:, :, off:off + 2],
                              vcol[:, 0:12].rearrange("p (t d) -> p t d", t=6))
            nc.sync.dma_start(v7_t[:, :, off:off + 2],
                              vcol[:, 0:6].rearrange("p (t d) -> p t d", t=3))

        pooled = cp.tile([128, 3, b_loc], F32, tag="pooled")
        nc4 = cp.tile([128, 1], F32, tag="nc4")
        nc.gpsimd.memset(nc4[:], -C4)
        nc7 = cp.tile([128, 1], F32, tag="nc7")
        nc.gpsimd.memset(nc7[:], -C7)

        MM = nc.tensor.matmul

        state = {}

        def h1_groups(pr):
            """Dense front half for pair (2pr, 2pr+1): mlp_in/V4 run both
            batches at N=512; Q/K per batch-half.  Returns (stage_a, stage_b)
            emit-thunk lists; stage_b must be emitted after the previous
            pair's B-half ST4 reads (slot-reuse order)."""
            bA = 2 * pr
            xfm = xp.tile([128, 2, 2, NA], F32R, tag="xfm")
            hfm = hp.tile([128, 6, 512], F32R, tag="hfm")
            qkt = [qkp.tile([128, 2, 2 * D2], F32R, tag="qkt4", name="qkt4")
                   for _ in range(2)]
            for h in range(2):
                state[bA + h] = dict(qkt=qkt[h], voff=260 * h)
            ga, gb = [], []

            def g_x():
                for h in range(2):
                    nc.sync.dma_start(
                        xfm[:, :, h, :],
                        xt[bA + h].rearrange("(t p) n -> p t n", p=128))
            ga.append(g_x)

            def g_mlp(d_t):
                p = ps_sm.tile([128, 512], F32, tag="psm")
                MM(p[:, 0:512], wit_t[:, 0, sl(d_t)], xfm[:, 0, :, :],
                   start=True, stop=False)
                MM(p[:, 0:512], wit_t[:, 1, sl(d_t)], xfm[:, 1, :, :],
                   start=False, stop=True)
                nc.vector.tensor_scalar_add(p[:, 0:512], p[:, 0:512],
                                            bcols_t[:, d_t:d_t + 1])
                silu2(hfm[:, d_t, :], p[:, 0:512], 512)
            for d_t in range(6):
                ga.append(lambda d_t=d_t: g_mlp(d_t))

            def g_qk(dst, h, n_t):
                ps = [ps_sm.tile([128, 512], F32, tag="psm", name="pqk")
                      for _ in range(3)]
                for c in range(3):
                    brow, o1 = qkbias[c]
                    MM(ps[c][:], o1[:, 0:128], brow[:],
                       start=True, stop=False)
                for e_t in range(6):
                    for c in range(3):  # same stationary 3x (ldw-opt)
                        MM(ps[c][:], hfm[:, e_t, sl(2 * h + n_t)],
                           aqkt_t[:, e_t, 512 * c:512 * (c + 1)],
                           start=False, stop=(e_t == 5))
                for c in range(3):
                    silu2(dst[:, n_t, 512 * c:512 * (c + 1)], ps[c][:], 512)
            for h, lst in ((0, ga), (1, gb)):
                for n_t in range(2):
                    lst.append(lambda a=qkt[h], hh=h, n=n_t: g_qk(a, hh, n))

            def g_v4(d_t):
                p = ps_sm.tile([128, 512], F32, tag="psm")
                for e_t in range(6):
                    MM(p[:, 0:512], avt_t[:, e_t, sl(d_t)], hfm[:, e_t, :],
                       start=(e_t == 0), stop=(e_t == 5))
                nc.vector.tensor_scalar_add(p[:, 0:512], p[:, 0:512],
                                            bcols_t[:, 6 + d_t:7 + d_t])
                t = tp.tile([128, 512], F32, tag="t")
                nc.scalar.activation(t[:], p[:, 0:512], AF.Tanh, scale=0.5)
                vo = v4_t[:, d_t, 0:520].rearrange(
                    "p (b x) -> p b x", x=260)[:, :, 0:256]
                nc.vector.scalar_tensor_tensor(
                    vo, t[:].rearrange("p (b x) -> p b x", x=256), 1.0,
                    p[:, 0:512].rearrange("p (b x) -> p b x", x=256),
                    OP.add, OP.mult)
            for d_t in range(6):
                gb.append(lambda d_t=d_t: g_v4(d_t))
            return ga, gb

        def h2_groups(pr):
            """Sparse back half for pair pr: per-batch attention4, then paired
            mlp4/h5/V7, per-batch block7 attention + pooling."""
            o4s = [o4p.tile([128, 512], F32R, tag="o4", name="o4")
                   for _ in range(6)]
            h5 = h5p.tile([128, 3, 512], F32R, tag="h5")
            qk7 = [[q7p.tile([128, 2, H], F32R, tag="qt7", name="qt7"),
                    k7p.tile([128, 2, H], F32R, tag="kt7", name="kt7")]
                   for _ in range(2)]
            sts = [state.pop(2 * pr), state.pop(2 * pr + 1)]

            # --- per-batch block4 attention ---
            def g_st(hb, e_t, et4, hc):
                qkt4 = sts[hb]["qkt"]
                p = ps_sm.tile([128, 384], F32, tag="psm")
                for n_t in range(2):
                    MM(p[:, 0:384],
                       qkt4[:, n_t, D2 + e_t * 128:D2 + (e_t + 1) * 128],
                       qkt4[:, n_t, hc:hc + 384],
                       start=(n_t == 0), stop=(n_t == 1))
                nc.scalar.activation(et4[:, e_t, :], p[:, 0:384], AF.Exp,
                                     bias=nc4[:, 0:1], scale=0.25)

            def g_av(hb, d_t, et4, half):
                voff = sts[hb]["voff"]
                d_g = half * 3 + d_t
                p = ps_sm.tile([128, 384], F32, tag="psm")
                for e_t in range(6):
                    MM(p[:, 0:258], et4[:, e_t, sl(d_t)],
                       v4_t[:, e_t, voff:voff + 258],
                       start=(e_t == 0), stop=(e_t == 5))
                rho = rp.tile([128, 1], F32, tag="rho")
                nc.vector.reciprocal(rho[:], p[:, 256:257])
                t = tp.tile([128, 512], F32, tag="t")
                nc.scalar.activation(t[:, 0:NA], p[:, 0:NA], AF.Tanh,
                                     scale=rho[:, 0:1])
                x2 = x2p.tile([128, NA], F32, tag="x2")
                nc.vector.tensor_scalar(x2[:], p[:, 0:NA], rho[:, 0:1],
                                        2.0, OP.mult, OP.mult)
                nc.vector.scalar_tensor_tensor(
                    o4s[d_g][:, 256 * hb:256 * hb + 256], t[:, 0:NA], 1.0,
                    x2[:], OP.add, OP.mult)

            for hb in range(2):
                et4s = [ep.tile([128, 6, 384], F32R, tag="et4", name="et4")
                        for _ in range(2)]
                for half in range(2):
                    for e_t in range(6):
                        yield (lambda hb=hb, e_t=e_t, et4=et4s[half],
                               hc=half * 384: g_st(hb, e_t, et4, hc))
                    for d_t in range(3):
                        yield (lambda hb=hb, d_t=d_t, et4=et4s[half],
                               half=half: g_av(hb, d_t, et4, half))

            # --- paired mlp4 -> h5 ---
            def g_mlp4(h_t):
                p = ps_sm.tile([128, 512], F32, tag="psm")
                for d_g in range(6):
                    MM(p[:], w4t_t[:, d_g, sl(h_t)], o4s[d_g][:],
                       start=(d_g == 0), stop=(d_g == 5))
                nc.vector.tensor_scalar_add(p[:], p[:],
                                            bcols_t[:, 12 + h_t:13 + h_t])
                silu2(h5[:, h_t, :], p[:], 512)
            for h_t in range(3):
                yield (lambda h_t=h_t: g_mlp4(h_t))

            # --- per-batch Q7/K7 (stationary = h5 pair slices) ---
            def g_qk7(hb, n_t):
                pq = ps_sm.tile([128, 384], F32, tag="psm", name="pq7")
                pk = ps_sm.tile([128, 384], F32, tag="psm", name="pk7")
                MM(pq[:, 0:H], ones_a[:, 0:128], bq7row[:, 0:H],
                   start=True, stop=False)
                MM(pk[:, 0:H], ones_b[:, 0:128], bk7row[:, 0:H],
                   start=True, stop=False)
                for h_t in range(3):  # same stationary 2x (ldw-opt)
                    MM(pq[:, 0:H], h5[:, h_t, sl(2 * hb + n_t)],
                       aq7t_t[:, h_t, 0:H], start=False, stop=(h_t == 2))
                    MM(pk[:, 0:H], h5[:, h_t, sl(2 * hb + n_t)],
                       ak7t_t[:, h_t, 0:H], start=False, stop=(h_t == 2))
                silu2(qk7[hb][0][:, n_t, :], pq[:, 0:H], H)
                silu2(qk7[hb][1][:, n_t, :], pk[:, 0:H], H)
            for hb in range(2):
                for n_t in range(2):
                    yield (lambda hh=hb, n=n_t: g_qk7(hh, n))

            # --- paired V7 ---
            def g_v7(d_t):
                p = ps_sm.tile([128, 512], F32, tag="psm")
                for h_t in range(3):
                    MM(p[:], av7t_t[:, h_t, sl(d_t)], h5[:, h_t, :],
                       start=(h_t == 0), stop=(h_t == 2))
                nc.vector.tensor_scalar_add(p[:], p[:],
                                            bcols_t[:, 15 + d_t:16 + d_t])
                t = tp.tile([128, 512], F32, tag="t")
                nc.scalar.activation(t[:], p[:], AF.Tanh, scale=0.5)
                vo = v7_t[:, d_t, 0:520].rearrange(
                    "p (b x) -> p b x", x=260)[:, :, 0:256]
                nc.vector.scalar_tensor_tensor(
                    vo, t[:].rearrange("p (b x) -> p b x", x=256), 1.0,
                    p[:].rearrange("p (b x) -> p b x", x=256),
                    OP.add, OP.mult)
            for d_t in range(3):
                yield (lambda d_t=d_t: g_v7(d_t))

            # --- per-batch block7 attention + pooling ---
            def g_st7(hb, e_t, et7):
                p = ps_sm.tile([128, 384], F32, tag="psm")
                for n_t in range(2):
                    MM(p[:, 0:H], qk7[hb][1][:, n_t, sl(e_t)],
                       qk7[hb][0][:, n_t, 0:H],
                       start=(n_t == 0), stop=(n_t == 1))
                nc.scalar.activation(et7[:, e_t, :], p[:, 0:H], AF.Exp,
                                     bias=nc7[:, 0:1], scale=0.25)

            def g_av7(hb, d_t, et7):
                voff = sts[hb]["voff"]
                b = 2 * pr + hb
                p = ps_sm.tile([128, 384], F32, tag="psm")
                for e_t in range(3):
                    MM(p[:, 0:258], et7[:, e_t, sl(d_t)],
                       v7_t[:, e_t, voff:voff + 258],
                       start=(e_t == 0), stop=(e_t == 2))
                rho = rp.tile([128, 1], F32, tag="rho")
                nc.vector.reciprocal(rho[:], p[:, 256:257])
                t = tp.tile([128, 512], F32, tag="t")
                nc.scalar.activation(t[:, 0:NA], p[:, 0:NA], AF.Tanh,
                                     scale=rho[:, 0:1])
                x2 = x2p.tile([128, NA], F32, tag="x2")
                nc.vector.tensor_scalar(x2[:], p[:, 0:NA], rho[:, 0:1], 2.0,
                                        OP.mult, OP.mult)
                nc.vector.scalar_tensor_tensor(
                    x2[:], t[:, 0:NA], 1.0, x2[:], OP.add, OP.mult,
                    accum_out=pooled[:, d_t, b:b + 1])

            for hb in range(2):
                et7 = ep.tile([128, 6, 384], F32R, tag="et4", name="et7")[:, 0:3, :]
                for e_t in range(3):
                    yield (lambda hb=hb, e_t=e_t, et7=et7: g_st7(hb, e_t, et7))
                for d_t in range(3):
                    yield (lambda hb=hb, d_t=d_t, et7=et7: g_av7(hb, d_t, et7))

        # Software-pipelined emission: pair pr's dense front half is
        # interleaved with pair pr-1's sparse back half (HAM duty), with
        # staging so every tile slot's readers precede its next writer in
        # per-engine emission order (else the scheduler deadlocks):
        #   stage A groups ride along g2[0:40]   (covers prev A's ST4)
        #   g2[40:49] emitted solo               (prev B's ST4)
        #   stage B groups ride along g2[49:]    (prev B's AV4 done early)
        def merge(g2part, g1part):
            i1 = i2 = 0
            n1, n2 = len(g1part), len(g2part)
            while i1 < n1 or i2 < n2:
                if i2 < n2 and (i1 >= n1 or i2 * n1 <= i1 * n2):
                    g2part[i2](); i2 += 1
                else:
                    g1part[i1](); i1 += 1

        n_pairs = b_loc // 2
        for pr in range(n_pairs + 1):
            ga, gb = h1_groups(pr) if pr < n_pairs else ([], [])
            g2 = list(h2_groups(pr - 1)) if pr >= 1 else []
            # stage so slot readers precede next writers in emission order:
            #  ga head (x, mlp_in) rides A-attn; qkA rides B-attn-head (needs
            #  prev A-ST4 done at g2[14]); qkB+V4 ride the tail (need prev
            #  B-ST4 at g2[32] / AV4 done at g2[35]).
            merge(g2[0:18], ga[0:7])
            merge(g2[18:33], ga[7:])
            for g in g2[33:36]:
                g()
            merge(g2[36:], gb)

        # ---------- final: out = silu(pooled^T Wo^T/(2*256) + bo) ----------
        wot_t = xp.tile([128, 3, OUT_DIM], F32, tag="xfm", name="wot")
        nc.sync.dma_start(wot_t[:], wot32.rearrange("(t p) d -> p t d", p=128))
        po = ps_sm.tile([b_loc, OUT_DIM], F32, tag="psm")
        for h_t in range(3):
            MM(po[:, :], pooled[:, h_t, :], wot_t[:, h_t, :],
               start=(h_t == 0), stop=False)
        MM(po[:, :], ones32_t[0:1, 0:b_loc], borow_t[0:1, 0:OUT_DIM],
           start=False, stop=True)
        out_sb = xp.tile([b_loc, OUT_DIM], F32, tag="xfm", name="out_sb")
        nc.scalar.activation(out_sb[:], po[:, :], AF.Silu)
        nc.sync.dma_start(out[:], out_sb[:])

    nc.compile()
    return nc


def _get_nc(b_loc=B_LOC):
    if b_loc not in _NC_CACHE:
        nc = bacc.Bacc("TRN2", target_bir_lowering=False, debug=False,
                       num_devices=NCORES)
        _NC_CACHE[b_loc] = build(nc, b_loc)
    return _NC_CACHE[b_loc]


def make_in_maps(inputs, b_loc=B_LOC):
    f = np.float32
    asT = lambda k: np.ascontiguousarray(np.asarray(inputs[k], f).T)
    row = lambda k, d: np.asarray(inputs[k], f).reshape(1, d)
    x = np.asarray(inputs["x"], f)
    def pack2(r0, r32, d):
        a = np.zeros((33, d), f)
        a[0] = r0.ravel(); a[32] = r32.ravel()
        return a
    bqk = np.concatenate([np.asarray(inputs["Bq4"], f).ravel(),
                          np.asarray(inputs["Bk4"], f).ravel()])
    brda = np.zeros((33, 1024), f)
    brda[0] = bqk[0:1024]
    brda[32, 0:512] = bqk[1024:1536]
    brh = np.zeros((33, H), f)
    brh[0] = np.asarray(inputs["Bq7"], f).ravel()
    brh[32] = np.asarray(inputs["Bk7"], f).ravel()
    ones2 = np.zeros((33, 128), f); ones2[0] = 1.0; ones2[32] = 1.0
    consts = dict(
        wit=asT("Wi"),
        aqkt=np.concatenate([asT("Aq4"), asT("Ak4")], axis=1) * 0.5,
        avt=asT("Av4") * 0.5,
        w4t=asT("W4") * 0.5,
        aq7t=asT("Aq7") * 0.5, ak7t=asT("Ak7") * 0.5, av7t=asT("Av7") * 0.5,
        wot32=asT("Wo") / (2.0 * NA),
        brda=brda, brh=brh, ones2=ones2,
        borow32=row("bo", OUT_DIM),
        ones32=np.ones((1, b_loc), f),
        vcol=np.full((128, 12), 4.0, f),
        bcols=np.concatenate([
            np.asarray(inputs["bi"], f).reshape(6, 128).T,
            np.asarray(inputs["Bv4"], f).reshape(6, 128).T,
            np.asarray(inputs["b4"], f).reshape(3, 128).T,
            np.asarray(inputs["Bv7"], f).reshape(3, 128).T], axis=1),
    )
    in_maps = []
    for c in range(NCORES):
        xb = x[c * b_loc:(c + 1) * b_loc]
        m = dict(consts)
        m["xt"] = np.ascontiguousarray(xb.transpose(0, 2, 1))
        in_maps.append(m)
    return in_maps


def kernel(**inputs) -> np.ndarray:
    nc = _get_nc()
    in_maps = make_in_maps(inputs)
    res = run_bass_kernel_spmd(nc, in_maps, core_ids=list(range(NCORES)))
    out = np.concatenate([r["out"] for r in res.results], axis=0)
    return out.reshape(B // NA, NA, OUT_DIM).astype(np.float32)
